# revision 37
# baseline (speedup 1.0000x reference)
"""Causal self-attention kernel for 8 Trainium2 NeuronCores.

Problem: B=4, T=2048, C=1024, NH=16, HD=64 (fp32 reference).

Sharding: core c = (batch b = c//2, head-group g = c%2 of 8 heads).
Per core, everything is computed in transposed layout so no on-device
transposes are needed:
  - host supplies xT = x[b].T in bf16 (pre-chunked [8, 128, T]), plus
    head-group-sliced/permuted bf16 weights (column-parallel W_attn,
    row-parallel W_proj), laid out so every DMA is contiguous
  - qT/kT [feat, tok] via W stationary / xT moving; v [tok, feat] via
    xT stationary / W_v moving, with a fused ones-column per head so the
    attention row-sum (softmax denominator) falls out of the same matmul
  - scores are computed transposed [keys, queries] per 128-key block;
    exp on ACT; diagonal blocks restrict the matmul/exp/AV free range to
    the causally-valid queries and mask only the [128,128] triangle;
    blocks entirely above the diagonal are skipped
  - y^T accumulates in PSUM over key blocks; the softmax denominator row
    is reciprocal'd on DVE, partition-broadcast on GPSIMD (no DRAM
    roundtrip), and multiplied into y
  - output projection is row-parallel -> partial out^T in bf16; pairwise
    ReduceScatter (+bias, added only on g=0 cores via host-zeroed bias)
    yields each core's final out^T rows; host concatenates + transposes.

All matmuls run bf16 (fp32 PSUM accumulation). Input DMAs ride the sync
queue ordered by first use; proj-partial stores + collectives ride the
gpsimd queue; final output stores ride the scalar queue so no compute
ever queues behind a collective.
"""

import numpy as np
from contextlib import ExitStack

import ml_dtypes
import concourse.bass as bass
import concourse.tile as tile
import concourse.mybir as mybir
from concourse import bacc
from concourse.bass_utils import run_bass_kernel_spmd

B, C, NH, HD = 4, 1024, 16, 64
NCORES = 8
NP = 4              # head pairs per core (8 heads)
QC = 512            # query-chunk (free dim of most matmuls)
KB = 128            # key block (partition dim of score blocks)
CCH = C // 128      # 8 contraction chunks
FP32 = mybir.dt.float32
BF16 = mybir.dt.bfloat16
EXP = mybir.ActivationFunctionType.Exp
GROUPS = [[0, 1], [2, 3], [4, 5], [6, 7]]
BF = ml_dtypes.bfloat16


def build_program(T=2048, mode="full"):
    nqc = T // QC
    nc = bacc.Bacc("TRN2", target_bir_lowering=False, debug=False,
                   num_devices=NCORES)

    xt_d = nc.dram_tensor("xt", [CCH, 128, T], BF16, kind="ExternalInput").ap()
    wqk_d = nc.dram_tensor("wqk", [CCH, 128, 8, 128], BF16, kind="ExternalInput").ap()
    bqk_d = nc.dram_tensor("bqk", [128, 8], FP32, kind="ExternalInput").ap()
    wv_d = nc.dram_tensor("wv", [CCH, 128, 512], BF16, kind="ExternalInput").ap()
    bv_d = nc.dram_tensor("bv", [8, HD + 1], FP32, kind="ExternalInput").ap()
    wp_d = nc.dram_tensor("wp", [NP, 128, 8, 128], BF16, kind="ExternalInput").ap()
    bp_d = nc.dram_tensor("bp", [128, 8], FP32, kind="ExternalInput").ap()
    tri_d = nc.dram_tensor("tri", [128, 2, 128], BF16, kind="ExternalInput").ap()
    out_d = nc.dram_tensor("out_t", [T // QC, 512, QC], BF16,
                       kind="ExternalOutput").ap()
    cc_in = [nc.dram_tensor(f"ccin{q}", [8, 128, QC], BF16).ap()
             for q in range(nqc)]
    cc_out = [nc.dram_tensor(f"ccout{q}", [4, 128, QC], BF16).ap()
              for q in range(nqc)]

    with tile.TileContext(nc) as tc, ExitStack() as ctx:
        resid = ctx.enter_context(tc.tile_pool(name="resid", bufs=1))
        xtp = ctx.enter_context(tc.tile_pool(name="xtp", bufs=2))
        qp = ctx.enter_context(tc.tile_pool(name="qp", bufs=2))
        yp = ctx.enter_context(tc.tile_pool(name="yp", bufs=2))
        ep = ctx.enter_context(tc.tile_pool(name="ep", bufs=4))
        sm = ctx.enter_context(tc.tile_pool(name="sm", bufs=3))
        op = ctx.enter_context(tc.tile_pool(name="op", bufs=2))
        ps_acc = ctx.enter_context(tc.tile_pool(name="ps_acc", bufs=2, space="PSUM"))
        ps_s = ctx.enter_context(tc.tile_pool(name="ps_s", bufs=2, space="PSUM"))
        ps_y = ctx.enter_context(tc.tile_pool(name="ps_y", bufs=2, space="PSUM"))

        # ---- residents. Transfer order ≈ trigger order, so put the
        # first-matmul gate (wv + xt chunk 0) at the head of the sync
        # queue and push everything else to gpsimd-triggered DMAs.
        bv_bc = resid.tile([128, 8, HD + 1], FP32, name="bv_bc")
        nc.sync.dma_start(out=bv_bc, in_=bv_d.partition_broadcast(128))
        wv_sb = resid.tile([128, CCH, 512], BF16, name="wv_sb")

        wqk_sb = resid.tile([128, CCH, 8, 128], BF16, name="wqk_sb")
        for cc in range(CCH):
            nc.gpsimd.dma_start(out=wqk_sb[:, cc], in_=wqk_d[cc])
        bqk_sb = resid.tile([128, 8], FP32, name="bqk_sb")
        nc.gpsimd.dma_start(out=bqk_sb, in_=bqk_d)

        wp_sb = resid.tile([128, NP, 8, 128], BF16, name="wp_sb")
        for p in range(NP):
            nc.gpsimd.dma_start(out=wp_sb[:, p], in_=wp_d[p])
        bp_sb = resid.tile([128, 8], FP32, name="bp_sb")
        nc.gpsimd.dma_start(out=bp_sb, in_=bp_d)
        # tri2[p, e, j] = 1.0 iff p <= j (same triangle for both heads)
        tri2 = resid.tile([128, 2, 128], BF16, name="tri2")
        nc.gpsimd.dma_start(out=tri2, in_=tri_d)

        ksb = [resid.tile([128, T], BF16, name=f"ksb{p}") for p in range(NP)]
        vsb = [resid.tile([128, 8, HD + 1], BF16, name=f"vsb{tb}")
               for tb in range(T // 128)]

        # ---------- emission helpers ----------
        def load_xt(qc):
            xt_sb = xtp.tile([128, CCH, QC], BF16, name="xt_sb")
            if qc == 0:
                # interleave wv/xt0 per-cc so the first emit_v matmuls
                # start as soon as each contraction chunk lands
                for cc in range(CCH):
                    nc.sync.dma_start(out=wv_sb[:, cc], in_=wv_d[cc])
                    nc.sync.dma_start(
                        out=xt_sb[:, cc], in_=xt_d[cc, :, 0:QC])
            else:
                nc.sync.dma_start(
                    out=xt_sb,
                    in_=xt_d[:, :, qc * QC:(qc + 1) * QC].rearrange("c p n -> p c n"))
            return xt_sb

        def emit_v(xt_sb, qc, j):
            tb = qc * (QC // 128) + j
            pv = ps_acc.tile([128, 512], FP32, name="pv")
            for cc in range(CCH):
                nc.tensor.matmul(
                    out=pv, lhsT=xt_sb[:, cc, j * 128:(j + 1) * 128],
                    rhs=wv_sb[:, cc], start=(cc == 0), stop=(cc == CCH - 1))
            nc.vector.tensor_copy(vsb[tb][:, :, HD:HD + 1],
                                  bv_bc[:, :, HD:HD + 1])
            nc.vector.tensor_add(
                vsb[tb][:, :, 0:HD],
                pv.rearrange("p (l d) -> p l d", l=8), bv_bc[:, :, 0:HD])

        def emit_qk(xt_sb, q_sb, qc, f):
            pqk = ps_acc.tile([128, QC], FP32, name="pqk", tag="pv")
            for cc in range(CCH):
                nc.tensor.matmul(
                    out=pqk, lhsT=wqk_sb[:, cc, f], rhs=xt_sb[:, cc],
                    start=(cc == 0), stop=(cc == CCH - 1))
            p, isk = f // 2, f % 2
            dst = (ksb[p][:, qc * QC:(qc + 1) * QC] if isk else q_sb[p])
            nc.vector.tensor_scalar_add(dst, pqk, bqk_sb[:, f:f + 1])

        def emit_proj(y_qc, po_t, qc, oc):
            pp = ps_acc.tile([128, QC], FP32, name="pp", tag="pv")
            for p in range(NP):
                nc.tensor.matmul(out=pp, lhsT=wp_sb[:, p, oc], rhs=y_qc[p],
                                 start=(p == 0), stop=(p == NP - 1))
            nc.vector.tensor_scalar_add(po_t[:, oc, :], pp, bp_sb[:, oc:oc + 1])
            if oc == 7:
                nc.gpsimd.dma_start(
                    out=cc_in[qc].rearrange("o p q -> p o q"), in_=po_t)

        def emit_rs(qc):
            if mode == "nors":
                nc.gpsimd.dma_start(out=cc_out[qc], in_=cc_in[qc][0:4])
            else:
                nc.gpsimd.collective_compute(
                    "ReduceScatter", mybir.AluOpType.add, replica_groups=GROUPS,
                    ins=[cc_in[qc]], outs=[cc_out[qc]])

        def emit_out(qc):
            # DRAM->DRAM stores, emitted only at the very end of the sync
            # queue: a store waits on its ReduceScatter, so queueing it
            # anywhere with compute behind it stalls that engine's queue
            nc.sync.dma_start(
                out=out_d[qc],
                in_=cc_out[qc].rearrange("o p q -> (o p) q"))

        def new_q():
            return [qp.tile([128, QC], BF16, name=f"qsb{p}", tag=f"qsb{p}")
                    for p in range(NP)]

        # ---------- prologue: chunk 0 qkv projection ----------
        xt_cur = load_xt(0)
        q_cur = new_q()
        for j in range(QC // 128):
            emit_v(xt_cur, 0, j)
        for f in range(8):
            emit_qk(xt_cur, q_cur, 0, f)

        y_saved = {}
        for qc in range(nqc):
            # background PE units interleaved into this chunk's attention:
            # next chunk's qkv projection + deferred output projections.
            # The last chunk is locally exp(ACT)-bound, so for nqc=4 both
            # proj(1) and proj(2) are deferred into it to keep the PE fed
            # (and HAM warm); each RS fires from the pacing stream right
            # after its input store.
            bg = []
            if qc + 1 < nqc:
                xt_nxt = load_xt(qc + 1)
                q_nxt = new_q()
                for j in range(QC // 128):
                    bg.append((emit_v, (xt_nxt, qc + 1, j)))
                for f in range(8):
                    bg.append((emit_qk, (xt_nxt, q_nxt, qc + 1, f)))
            else:
                xt_nxt, q_nxt = None, None
            for pj in ([qc - 1] if qc >= 1 else []):
                y_pj = y_saved.pop(pj)
                po_t = op.tile([128, 8, QC], BF16, name="po_t", tag="po_t")
                for oc in range(8):
                    bg.append((emit_proj, (y_pj, po_t, pj, oc)))
            bg_total = len(bg)

            njb = 4 * (qc + 1)
            steps = NP * njb

            y_cur = [yp.tile([128, QC], BF16, name=f"y{p}", tag=f"y{p}")
                     for p in range(NP)]
            step = 0
            for p in range(NP):
                yps = [ps_y.tile([HD + 1, QC], FP32, name=f"yps{e}", tag="yps")
                       for e in (0, 1)]
                for jb in range(njb):
                    r = jb - 4 * qc
                    off = 128 * r if r > 0 else 0
                    sps = ps_s.tile([128, 2, QC], FP32, name="sps")
                    for e in (0, 1):
                        nc.tensor.matmul(
                            out=sps[:, e, off:],
                            lhsT=ksb[p][e * HD:(e + 1) * HD,
                                        jb * KB:(jb + 1) * KB],
                            rhs=q_cur[p][e * HD:(e + 1) * HD, off:],
                            start=True, stop=True)
                    esb = ep.tile([128, 2, QC], BF16, name="esb")
                    nc.scalar.activation(out=esb[:, :, off:], in_=sps[:, :, off:],
                                         func=EXP, scale=0.125)
                    if r >= 0:
                        nc.vector.tensor_mul(
                            esb[:, :, off:off + 128],
                            esb[:, :, off:off + 128], tri2)
                    for e in (0, 1):
                        nc.tensor.matmul(
                            out=yps[e][:, off:], lhsT=vsb[jb][:, 2 * p + e, :],
                            rhs=esb[:, e, off:],
                            start=(jb == 0), stop=(jb == njb - 1))
                    # keep the in-order PE stream dense: spread background
                    # units evenly across the attention steps
                    step += 1
                    while bg and len(bg) > bg_total * (steps - step) // steps:
                        fn, args = bg.pop(0)
                        fn(*args)
                # normalize: 1/Z on DVE from the PSUM ones-row, partition-
                # broadcast on GPSIMD, multiply into y (no DRAM roundtrip)
                for e in (0, 1):
                    rz = sm.tile([1, QC], FP32, name="rz", tag="rz")
                    zb = sm.tile([HD, QC], FP32, name="zb", tag="zb")
                    # partition-shifted custom-DVE ops NaN on HW: plain
                    # shifted copy first, then aligned reciprocal
                    nc.vector.tensor_copy(rz, yps[e][HD:HD + 1, :])
                    nc.vector.reciprocal_approx_fast(rz, rz)
                    nc.gpsimd.partition_broadcast(zb, rz, channels=HD)
                    nc.vector.tensor_mul(
                        y_cur[p][e * HD:(e + 1) * HD, :], yps[e][0:HD, :], zb)
            for fn, args in bg:
                fn(*args)
            if qc > 0:
                emit_rs(qc - 1)
            y_saved[qc] = y_cur
            xt_cur, q_cur = xt_nxt, q_nxt

        # epilogue: last chunk's projection + reduce-scatter
        po_t = op.tile([128, 8, QC], BF16, name="po_t", tag="po_t")
        for oc in range(8):
            emit_proj(y_saved[nqc - 1], po_t, nqc - 1, oc)
        emit_rs(nqc - 1)
        for qc in range(nqc):
            emit_out(qc)

    nc.compile()
    return nc


def shard_inputs(x, W_attn, b_attn, W_proj, b_proj):
    T = x.shape[1]
    in_maps = []
    tri = (np.arange(128)[:, None] <= np.arange(128)[None, :])
    tri2 = np.repeat(tri[:, None, :], 2, axis=1).astype(BF)
    for c in range(NCORES):
        b, g = c // 2, c % 2
        xt = np.ascontiguousarray(x[b].T).astype(BF).reshape(CCH, 128, T)
        # w_qk columns: feat chunk f = 2p+isK holds q (isK=0) or k (isK=1)
        # features of heads (8g+2p, 8g+2p+1)
        qk_idx = []
        for f in range(8):
            p, isk = f // 2, f % 2
            for e in (0, 1):
                h = 8 * g + 2 * p + e
                base = isk * C + h * HD
                qk_idx.append(np.arange(base, base + HD))
        qk_idx = np.concatenate(qk_idx)
        v_idx = np.concatenate(
            [np.arange(2 * C + (8 * g + l) * HD, 2 * C + (8 * g + l) * HD + HD)
             for l in range(8)])
        p_idx = np.concatenate(
            [np.arange((8 * g + l) * HD, (8 * g + l) * HD + HD)
             for l in range(8)])
        bv = np.ones((8, HD + 1), np.float32)
        bv[:, 0:HD] = b_attn[v_idx].reshape(8, HD)
        in_maps.append({
            "tri": tri2,
            "xt": xt,
            "wqk": np.ascontiguousarray(W_attn[:, qk_idx]).astype(BF)
                     .reshape(CCH, 128, 8, 128),
            "bqk": np.ascontiguousarray(
                b_attn[qk_idx].reshape(8, 128).T.astype(np.float32)),
            "wv": np.ascontiguousarray(W_attn[:, v_idx]).astype(BF)
                    .reshape(CCH, 128, 512),
            "bv": bv,
            "wp": np.ascontiguousarray(W_proj[p_idx, :]).astype(BF)
                    .reshape(NP, 128, 8, 128),
            "bp": np.ascontiguousarray(
                (b_proj if g == 0 else np.zeros(C, np.float32))
                .reshape(8, 128).T.astype(np.float32)),
        })
    return in_maps


def assemble_output(results, T):
    out = np.empty((B, T, C), np.float32)
    for b in range(B):
        # out_t is chunk-major [nqc, 512, QC] so device stores are contiguous
        top = np.asarray(results[2 * b]["out_t"]).astype(np.float32)
        bot = np.asarray(results[2 * b + 1]["out_t"]).astype(np.float32)
        for qc in range(T // QC):
            out[b, qc * QC:(qc + 1) * QC, 0:512] = top[qc].T
            out[b, qc * QC:(qc + 1) * QC, 512:1024] = bot[qc].T
    return out


_PROG = {}


def _get_program(T, mode="full"):
    key = (T, mode)
    if key not in _PROG:
        _PROG[key] = build_program(T, mode)
    return _PROG[key]


def run_sharded(inputs, trace=False, mode="full"):
    """Returns (output [B,T,C], BassKernelResults)."""
    x = np.asarray(inputs["x"])
    T = x.shape[1]
    nc = _get_program(T, mode)
    in_maps = shard_inputs(x, np.asarray(inputs["W_attn"]),
                           np.asarray(inputs["b_attn"]),
                           np.asarray(inputs["W_proj"]),
                           np.asarray(inputs["b_proj"]))
    res = run_bass_kernel_spmd(nc, in_maps, list(range(NCORES)), trace=trace)
    return assemble_output(res.results, T), res


def kernel(**inputs):
    out, _ = run_sharded(inputs)
    return out
